# revision 1
# baseline (speedup 1.0000x reference)
"""CenterLoss on 8 TRN2 NeuronCores.

loss = mean_i clip(||x_i - centers[labels_i]||^2, 1e-12, 1e12)

Strategy (data-parallel, per sharding hint):
 - shard x/labels along batch: 4096 rows per core; centers (200MB) replicated.
 - per core: load the x shard into SBUF once (8MB), gather the 4096 needed
   center rows with indirect DMA (128 rows / 2KB each per instruction),
   diff on DVE, square+row-sum fused on the scalar engine (ACT accum_out),
   final [128,1] per-partition partial sums DMA'd out.
 - host: sum the 8x128 partials, divide by B.
"""

import numpy as np

import concourse.bacc as bacc
import concourse.bass as bass
import concourse.mybir as mybir
import concourse.tile as tile
from concourse.bass_utils import run_bass_kernel_spmd

B = 32768
F = 512
C = 100000
NCORES = 8
BPC = B // NCORES  # 4096 rows per core
P = 128
T = BPC // P  # 32 column-tiles per core

f32 = mybir.dt.float32
i32 = mybir.dt.int32


def build(bpc: int = BPC, feat: int = F, ncls: int = C) -> bass.Bass:
    t_tiles = bpc // P
    nc = bacc.Bacc(None, target_bir_lowering=False)
    x = nc.declare_dram_parameter("x", [bpc, feat], f32, isOutput=False)
    labels = nc.declare_dram_parameter("labels", [bpc], i32, isOutput=False)
    centers = nc.declare_dram_parameter("centers", [ncls, feat], f32, isOutput=False)
    out = nc.declare_dram_parameter("out", [P, 1], f32, isOutput=True)

    chunk = min(4, t_tiles)  # tiles per x-load chunk (1MB per dma_start)
    n_chunks = (t_tiles + chunk - 1) // chunk
    with tile.TileContext(nc) as tc:
        with (
            tc.tile_pool(name="big", bufs=1) as big,
            tc.tile_pool(name="xc", bufs=3) as xc,
            tc.tile_pool(name="cg", bufs=8) as cg,
            tc.tile_pool(name="work", bufs=8) as work,
        ):
            # x viewed as [P, t_tiles, feat] with row index p*t_tiles + t:
            # contiguous per partition; loaded in 1MB chunks so each compute
            # tile waits on a single DMA semaphore.
            xv = x[:].rearrange("(p t) f -> p t f", p=P)
            lab = big.tile([P, t_tiles], i32)
            acc = big.tile([P, t_tiles], f32)
            nc.sync.dma_start(
                out=lab[:], in_=labels[:].rearrange("(p t) -> p t", p=P)
            )
            for ci in range(n_chunks):
                t0 = ci * chunk
                t1 = min(t0 + chunk, t_tiles)
                nt = t1 - t0
                x_chunk = xc.tile([P, chunk * feat], f32, tag="x")
                nc.sync.dma_start(
                    out=x_chunk[:, : nt * feat],
                    in_=xv[:, t0:t1, :].rearrange("p t f -> p (t f)"),
                )
                # NOTE: the HW indirect-DMA ucode consumes ONE offset per dest
                # partition row and streams the rest contiguously (CoreSim's
                # flat multi-offset model does NOT match HW) — so each gather
                # must be [P, feat] with a [P, 1] offset column.
                for j in range(nt):
                    t = t0 + j
                    c_tile = cg.tile([P, feat], f32, tag="c")
                    diff = work.tile([P, feat], f32, tag="d")
                    sq = work.tile([P, feat], f32, tag="s")
                    nc.gpsimd.indirect_dma_start(
                        out=c_tile[:],
                        out_offset=None,
                        in_=centers[:],
                        in_offset=bass.IndirectOffsetOnAxis(
                            ap=lab[:, t : t + 1], axis=0
                        ),
                    )
                    nc.vector.tensor_tensor(
                        out=diff[:],
                        in0=x_chunk[:, j * feat : (j + 1) * feat],
                        in1=c_tile[:],
                        op=mybir.AluOpType.subtract,
                    )
                    nc.scalar.activation(
                        out=sq[:],
                        in_=diff[:],
                        func=mybir.ActivationFunctionType.Square,
                        accum_out=acc[:, t : t + 1],
                    )
            # clamp per-row dist like the reference, then sum the row dists
            accv = big.tile([P, 1], f32)
            nc.vector.tensor_scalar(
                out=acc[:],
                in0=acc[:],
                scalar1=1e-12,
                scalar2=1e12,
                op0=mybir.AluOpType.max,
                op1=mybir.AluOpType.min,
            )
            nc.vector.tensor_reduce(
                out=accv[:],
                in_=acc[:],
                axis=mybir.AxisListType.X,
                op=mybir.AluOpType.add,
            )
            nc.sync.dma_start(out=out[:], in_=accv[:])
    nc.finalize()
    return nc


def kernel(x, labels, centers):
    nc = build()
    xs = np.ascontiguousarray(np.asarray(x, dtype=np.float32))
    labs = np.ascontiguousarray(np.asarray(labels).astype(np.int32))
    cens = np.ascontiguousarray(np.asarray(centers, dtype=np.float32))
    in_maps = []
    for k in range(NCORES):
        sl = slice(k * BPC, (k + 1) * BPC)
        in_maps.append(
            {
                "x": np.ascontiguousarray(xs[sl]),
                "labels": np.ascontiguousarray(labs[sl]),
                "centers": cens,
            }
        )
    res = run_bass_kernel_spmd(nc, in_maps, core_ids=list(range(NCORES)))
    total = sum(float(np.sum(r["out"], dtype=np.float64)) for r in res.results)
    return np.asarray(total / B, dtype=np.float32)



# revision 2
# speedup vs baseline: 1.0977x; 1.0977x over previous
"""CenterLoss on 8 TRN2 NeuronCores — v3.

loss = mean_i clip(||x_i - centers[labels_i]||^2, 1e-12, 1e12)

v1 (77.7us) was DMA-bound: 16MB/core of f32 at ~340GB/s.
v2 (80.4us) cut traffic 4x with fp8 but dma_gather's Q7 descriptor
generation (8.5ns/row + 14us library load) became the serial bottleneck.

v3: fp8 traffic (4MB/core) + batch-sorted labels + v1's per-block
indirect_dma_start gathers (128 rows each, ~1us fixed Pool cost, no Q7
library needed; HW-probed: the ucode consumes exactly one offset per
partition, so 128 rows/instruction is a hard ceiling). Sorted labels
make each 128-row block's gather addresses a ~400-class window in HBM.

Host staging (sharding-strategy choices, all content-preserving):
 - sort batch rows by label (mean is permutation-invariant), 4096/core
 - per core: rebase labels to the shard's 32768-class centers window
 - x row t*128+p staged at partition p, block t (matches gather layout)
 - x/centers cast to fp8e4m3 (rel err ~7e-4, tolerance 2e-2)
"""

import numpy as np

import concourse.bacc as bacc
import concourse.bass as bass
import concourse.mybir as mybir
import concourse.tile as tile
from concourse.bass_utils import run_bass_kernel_spmd

B = 32768
F = 512
C = 100000
NCORES = 8
BPC = B // NCORES  # 4096 rows per core
P = 128
G = BPC // P  # 32 row-blocks of [128, F] per core
CSLICE = 32768  # per-core centers window (fits index in window)
K = 8  # row-blocks gathered per indirect DMA instruction
NCH = G // K  # gather/x chunks

f32 = mybir.dt.float32
i32 = mybir.dt.int32
bf16 = mybir.dt.bfloat16
DT = mybir.dt.float8e4
NP_DT = mybir.dt.np(DT)


def build() -> bass.Bass:
    # 4x the SWDGE descriptor ring: the gather stream otherwise stalls
    # descriptor generation on ring drain (~0.4us/gather).
    nc = bacc.Bacc(None, target_bir_lowering=False, dynamic_dma_scratch_size=65536)
    x = nc.declare_dram_parameter("x", [P, G * F], DT, isOutput=False)
    idx = nc.declare_dram_parameter("idx", [P, G], i32, isOutput=False)
    centers = nc.declare_dram_parameter("centers", [CSLICE, F], DT, isOutput=False)
    out = nc.declare_dram_parameter("out", [P, 1], f32, isOutput=True)

    with tile.TileContext(nc) as tc:
        with (
            tc.tile_pool(name="big", bufs=1) as big,
            tc.tile_pool(name="xc", bufs=3) as xc,
            tc.tile_pool(name="cg", bufs=8) as cg,
            tc.tile_pool(name="work", bufs=8) as work,
        ):
            lab = big.tile([P, G], i32)
            nc.sync.dma_start(out=lab[:], in_=idx[:])
            acc = big.tile([P, G], f32)
            for ci in range(NCH):
                xch = xc.tile([P, K * F], DT, tag="x")
                nc.sync.dma_start(
                    out=xch[:], in_=x[:, ci * K * F : (ci + 1) * K * F]
                )
                for j in range(K):
                    t = ci * K + j
                    cch = cg.tile([P, F], DT, tag="c")
                    diff = work.tile([P, F], bf16, tag="d")
                    sq = work.tile([P, F], bf16, tag="s")
                    nc.gpsimd.indirect_dma_start(
                        out=cch[:],
                        out_offset=None,
                        in_=centers[:],
                        in_offset=bass.IndirectOffsetOnAxis(
                            ap=lab[:, t : t + 1], axis=0
                        ),
                    )
                    nc.vector.tensor_tensor(
                        out=diff[:],
                        in0=xch[:, j * F : (j + 1) * F],
                        in1=cch[:],
                        op=mybir.AluOpType.subtract,
                    )
                    nc.scalar.activation(
                        out=sq[:],
                        in_=diff[:],
                        func=mybir.ActivationFunctionType.Square,
                        accum_out=acc[:, t : t + 1],
                    )
            accv = big.tile([P, 1], f32)
            nc.vector.tensor_scalar(
                out=acc[:],
                in0=acc[:],
                scalar1=1e-12,
                scalar2=1e12,
                op0=mybir.AluOpType.max,
                op1=mybir.AluOpType.min,
            )
            nc.vector.tensor_reduce(
                out=accv[:],
                in_=acc[:],
                axis=mybir.AxisListType.X,
                op=mybir.AluOpType.add,
            )
            nc.sync.dma_start(out=out[:], in_=accv[:])
    nc.finalize()
    return nc


def make_in_maps(x, labels, centers):
    xs = np.asarray(x, dtype=np.float32)
    labs = np.asarray(labels).astype(np.int64)
    cens = np.asarray(centers, dtype=np.float32)
    order = np.argsort(labs, kind="stable")
    xs_s = xs[order]
    ls = labs[order]
    cens_q = cens.astype(NP_DT)
    in_maps = []
    for k in range(NCORES):
        sl = slice(k * BPC, (k + 1) * BPC)
        lsh = ls[sl]
        base = min(int(lsh[0]), C - CSLICE)
        rel = lsh - base
        assert rel.min() >= 0 and rel.max() < CSLICE, (
            f"shard {k} label span {rel.max()} exceeds centers window"
        )
        # x row t*128+p -> partition p, block t (128 consecutive sorted
        # labels per gather block: tight HBM window per instruction)
        idx_np = rel.astype(np.int32).reshape(G, P).T
        x_np = (
            xs_s[sl]
            .astype(NP_DT)
            .reshape(G, P, F)
            .transpose(1, 0, 2)
            .reshape(P, G * F)
        )
        in_maps.append(
            {
                "x": np.ascontiguousarray(x_np),
                "idx": np.ascontiguousarray(idx_np),
                "centers": np.ascontiguousarray(cens_q[base : base + CSLICE]),
            }
        )
    return in_maps


def kernel(x, labels, centers):
    nc = build()
    in_maps = make_in_maps(x, labels, centers)
    res = run_bass_kernel_spmd(nc, in_maps, core_ids=list(range(NCORES)))
    total = sum(float(np.sum(r["out"], dtype=np.float64)) for r in res.results)
    return np.asarray(total / B, dtype=np.float32)
